# revision 21
# baseline (speedup 1.0000x reference)
"""Causal multi-head self-attention with RoPE on 8 TRN2 NeuronCores.

Problem (hardcoded): B=2, S=2048, D=1024, H=16, d_k=64, fp32 I/O.

Sharding (data + tensor parallel): core c -> batch c//4, head group c%4
(4 heads = 256 dims). Wq/Wk/Wv column-split, Wo row-split; host sums the
4 partial [S, D] outputs per batch (bf16 partials, fp32 accumulation).

v3 structure:
  - All 4 heads projected chunk-by-chunk (512 tokens); attention column j
    streams right after chunk j.
  - Attention inner loop software-pipelined: score pair for sk-tile i is
    emitted together with the P@V pair for tile i-1, so the PE stream is
    [S S PV PV] per group and exp (ACT) overlaps the PE work.
  - Projection / V / output-projection pieces are interleaved between
    attention groups as PE filler (keeps PE warm through exp stalls and
    column transitions).
  - Score matmuls (contraction 64) issued as adjacent pairs on PE row
    groups 0/64 so they co-execute on the tiled PE array.
  - RoPE rotate-half partner lives 16 rows away inside each 32-partition
    quadrant (host-side weight row permutation): partner tensor comes from
    a DVE stream_shuffle; cos/sin multiplies are fused into the PSUM
    eviction (DVE tensor ops reading PSUM); final add on GpSimd.
  - Softmax denominator rides as the 65th output row of P@V (ones column
    in V); triangular causal mask applied post-exp on GpSimd.
  - Output written bf16 (halves the output DMA); host sums in fp32.
"""

import numpy as np
import ml_dtypes

B, S, D = 2, 2048, 1024
H, DK = 16, 64
HPC = 4          # heads per core
E = HPC * DK     # 256 output dims per core
P = 128
KS = D // P      # 8 contraction subtiles
SC = 512         # s-chunk width
NSC = S // SC    # 4 chunks (also the 4 attention columns)
NSK = S // P     # 16 sk tiles
BF = ml_dtypes.bfloat16
SHUF16 = list(range(16, 32)) + list(range(0, 16))

_CACHE = {}


def _build_nc():
    import concourse.bacc as bacc
    import concourse.mybir as mybir
    import concourse.tile as tile
    from contextlib import ExitStack

    bf = mybir.dt.bfloat16
    f32 = mybir.dt.float32
    Exp = mybir.ActivationFunctionType.Exp

    nc = bacc.Bacc("TRN2", target_bir_lowering=False)

    xT = nc.dram_tensor("xT", [D, S], bf, kind="ExternalInput")
    wq = nc.dram_tensor("wq", [D, E], bf, kind="ExternalInput")
    wk = nc.dram_tensor("wk", [D, E], bf, kind="ExternalInput")
    wv = nc.dram_tensor("wv", [D, E], bf, kind="ExternalInput")
    wo = nc.dram_tensor("wo", [E, D], bf, kind="ExternalInput")
    cs = nc.dram_tensor("cs", [P, S], bf, kind="ExternalInput")
    sn = nc.dram_tensor("sn", [P, S], bf, kind="ExternalInput")
    tri = nc.dram_tensor("tri", [P, P], bf, kind="ExternalInput")
    out = nc.dram_tensor("out", [S, D], bf, kind="ExternalOutput")

    from concourse import library_config

    with tile.TileContext(nc) as tc, ExitStack() as ctx:
        # one gpsimd ucode library for tensor_tensor AND partition_broadcast:
        # avoids mid-kernel library reload thrash on the Pool engine
        try:
            nc.gpsimd.load_library(library_config.proxy)
        except Exception:
            pass
        const = ctx.enter_context(tc.tile_pool(name="const", bufs=1))
        work = ctx.enter_context(tc.tile_pool(name="work", bufs=3))
        pexp_pool = ctx.enter_context(tc.tile_pool(name="pexpp", bufs=6))
        pa = ctx.enter_context(tc.tile_pool(name="pa", bufs=2, space="PSUM"))
        pb = ctx.enter_context(tc.tile_pool(name="pb", bufs=2, space="PSUM"))
        pvp = ctx.enter_context(tc.tile_pool(name="pvp", bufs=1, space="PSUM"))

        # ---- input DMAs: wq + chunk 0 first so compute starts early ----
        xTv = xT.rearrange("(ks p) s -> p ks s", p=P)
        wqv = wq.rearrange("(ks p) e -> p ks e", p=P)
        wq_sb = const.tile([P, KS, E], bf, tag="wq")
        nc.sync.dma_start(wq_sb[:], wqv[:])
        xss = []
        for st in range(NSC):
            xc = const.tile([P, KS, SC], bf, tag=f"xs{st}", name=f"xs{st}")
            xss.append(xc)
        nc.sync.dma_start(xss[0][:, :, 0:SC // 2], xTv[:, :, 0:SC // 2])
        cs_sb = const.tile([P, S], bf, tag="cs")
        sn_sb = const.tile([P, S], bf, tag="sn")
        nc.sync.dma_start(cs_sb[:, 0:SC], cs[:, 0:SC])
        nc.sync.dma_start(sn_sb[:, 0:SC], sn[:, 0:SC])
        nc.sync.dma_start(xss[0][:, :, SC // 2:SC], xTv[:, :, SC // 2:SC])
        wk_sb = const.tile([P, KS, E], bf, tag="wk")
        nc.sync.dma_start(wk_sb[:], wk.rearrange("(ks p) e -> p ks e", p=P))
        nc.sync.dma_start(cs_sb[:, SC:], cs[:, SC:])
        nc.sync.dma_start(sn_sb[:, SC:], sn[:, SC:])
        wv_sb = const.tile([P, KS, E], bf, tag="wv")
        nc.sync.dma_start(wv_sb[:], wv.rearrange("(ks p) e -> p ks e", p=P))
        tri_sb = const.tile([P, P], bf, tag="tri")
        nc.sync.dma_start(tri_sb[:], tri[:])
        nc.sync.dma_start(xss[1][:], xTv[:, :, SC:2 * SC])
        wo_sb = const.tile([P, 2, D], bf, tag="wo")
        nc.sync.dma_start(wo_sb[:], wo.rearrange("(ks p) e -> p ks e", p=P))
        nc.sync.dma_start(xss[2][:], xTv[:, :, 2 * SC:3 * SC])
        nc.sync.dma_start(xss[3][:], xTv[:, :, 3 * SC:4 * SC])

        # persistent tensors
        qts = [const.tile([P, S], bf, tag=f"qt{eb}", name=f"qt{eb}") for eb in range(2)]
        kts = [const.tile([P, S], bf, tag=f"kt{eb}", name=f"kt{eb}") for eb in range(2)]
        # V augmented with a ones column per head: [s-part, sk-tile, 4*(64+1)]
        vaug = const.tile([P, NSK, HPC * (DK + 1)], bf, tag="vaug")
        vaug4 = vaug.rearrange("p t (h e) -> p t h e", h=HPC)
        nc.vector.memset(vaug4[:, :, :, DK], 1.0)
        # normalized attention values, laid out as Wo lhsT [e%128, e//128, s]
        vals = const.tile([P, 2, S], bf, tag="vals")

        def proj_qk_piece(w_sb, eb, sc, dst, half=None):
            """dst[:, chunk sc] = rope(W.T @ x.T) for e-block eb.

            half=0/1 processes a 256-column sub-chunk (shorter rope latency
            for the startup-critical chunk 0)."""
            if half is None:
                lo, w = 0, SC
            else:
                lo, w = half * (SC // 2), SC // 2
            cols = slice(sc * SC + lo, sc * SC + lo + w)
            ps = pa.tile([P, SC], f32, tag="pa")
            for ks in range(KS):
                nc.tensor.matmul(
                    ps[:, 0:w],
                    lhsT=w_sb[:, ks, eb * P:(eb + 1) * P],
                    rhs=xss[sc][:, ks, lo:lo + w],
                    start=(ks == 0), stop=(ks == KS - 1),
                )
            t = work.tile([P, SC], bf, tag="t")
            v = work.tile([P, SC], bf, tag="v")
            if half == 1:
                # alternate rope pipeline through ACT+GpSimd so startup
                # chunk-0 pieces don't all serialize on the DVE
                q0 = work.tile([P, SC], bf, tag="q0")
                nc.scalar.copy(out=q0[:, 0:w], in_=ps[:, 0:w])
                nc.gpsimd.tensor_mul(out=t[:, 0:w], in0=q0[:, 0:w],
                                     in1=cs_sb[:, cols])
                nc.gpsimd.tensor_mul(out=v[:, 0:w], in0=q0[:, 0:w],
                                     in1=sn_sb[:, cols])
            else:
                nc.vector.tensor_mul(out=t[:, 0:w], in0=ps[:, 0:w],
                                     in1=cs_sb[:, cols])
                nc.vector.tensor_mul(out=v[:, 0:w], in0=ps[:, 0:w],
                                     in1=sn_sb[:, cols])
            u = work.tile([P, SC], bf, tag="u")
            nc.vector.stream_shuffle(out=u[:, 0:w], in_=v[:, 0:w], mask=SHUF16)
            nc.gpsimd.tensor_add(out=dst[:, cols], in0=t[:, 0:w], in1=u[:, 0:w])

        def proj_v_piece(sc, s4):
            """V for one 128-token block into its vaug slot."""
            sst = sc * 4 + s4
            ps = pa.tile([P, SC], f32, tag="pa")
            pv256 = ps[:, 0:E]
            for ks in range(KS):
                nc.tensor.matmul(
                    pv256,
                    lhsT=xss[sc][:, ks, s4 * P:(s4 + 1) * P],
                    rhs=wv_sb[:, ks, :],
                    start=(ks == 0), stop=(ks == KS - 1),
                )
            nc.vector.tensor_copy(
                out=vaug4[:, sst, :, 0:DK],
                in_=pv256.rearrange("p (h e) -> p h e", h=HPC),
            )

        def outproj_piece(j, s4, tail):
            """out[sq block, :] = vals.T @ woT for 128 tokens."""
            sq = j * 4 + s4
            for n2 in range(2):
                ps = pa.tile([P, SC], f32, tag="pa")
                for ks2 in range(2):
                    nc.tensor.matmul(
                        ps[:],
                        lhsT=vals[:, ks2, sq * P:(sq + 1) * P],
                        rhs=wo_sb[:, ks2, n2 * SC:(n2 + 1) * SC],
                        start=(ks2 == 0), stop=(ks2 == 1),
                    )
                ostg = work.tile([P, SC], bf, tag="ostg", name="ostg")
                if tail and n2 == 1:
                    nc.scalar.copy(out=ostg[:], in_=ps[:])
                else:
                    nc.vector.tensor_copy(out=ostg[:], in_=ps[:])
                nc.sync.dma_start(
                    out=out[sq * P:(sq + 1) * P, n2 * SC:(n2 + 1) * SC],
                    in_=ostg[:])

        def attention_col(j, hp, filler, fill_every, tail=False):
            """Attention for sq column j, head pair hp (heads 2hp, 2hp+1).

            Emits score pair i together with P@V pair i-1 (one-group software
            pipeline); pulls a filler piece every `fill_every` groups.
            """
            jcols = slice(j * SC, (j + 1) * SC)
            ntiles = 4 * j + 4
            pvts = [pvp.tile([P, SC], f32, tag=f"pv{par}", name=f"pv{par}")
                    for par in range(2)]

            def emit_pv(prev):
                pexp, off, i = prev
                for par in range(2):
                    hl = 2 * hp + par
                    nc.tensor.matmul(
                        pvts[par][0:DK + 1, off:],
                        lhsT=vaug[:, i, hl * (DK + 1):(hl + 1) * (DK + 1)],
                        rhs=pexp[:, par, off:],
                        start=(i == 0), stop=(i == ntiles - 1),
                    )

            pend = []
            for i in range(ntiles):
                c = i - 4 * j
                off = c * P if c > 0 else 0
                stp = pb.tile([P, 2, SC], f32, tag="pb", name="stp")
                for par in range(2):
                    nc.tensor.matmul(
                        stp[:, par, off:],
                        lhsT=kts[hp][64 * par:64 * par + 64, i * P:(i + 1) * P],
                        rhs=qts[hp][64 * par:64 * par + 64,
                                    j * SC + off:(j + 1) * SC],
                        start=True, stop=True,
                    )
                if len(pend) >= 2:
                    emit_pv(pend.pop(0))
                pexp = pexp_pool.tile([P, 2, SC], bf, tag="pexp", name="pexp")
                nc.scalar.activation(out=pexp[:, :, off:], in_=stp[:, :, off:],
                                     func=Exp)
                if c >= 0:  # diagonal subtile: triangular mask
                    for par in range(2):
                        sl = pexp[:, par, c * P:(c + 1) * P]
                        nc.gpsimd.tensor_mul(out=sl, in0=sl, in1=tri_sb[:])
                pend.append((pexp, off, i))
                # back-load filler pieces: the PE starves at the column tail
                # (PV flush + normalize), not at the head
                if filler:
                    start = max(0, ntiles - 2 * len(filler))
                    if i >= start and (i - start) % 2 == 1:
                        filler.pop(0)()
            while pend:
                emit_pv(pend.pop(0))

            # normalize by the softmax denominator (row DK of pvts).
            # tail=True: 256-column slices with the final output projection
            # interleaved, shortening the end-of-kernel serial chain.
            slices = (0, 1) if tail else (None,)
            for sl in slices:
                lo = 0 if sl is None else sl * (SC // 2)
                w = SC if sl is None else SC // 2
                cw = slice(lo, lo + w)
                for par in range(2):
                    rsb = work.tile([P, SC], f32, tag="rsb")
                    rb = work.tile([P, SC], f32, tag="rb")
                    lsb = work.tile([P, SC], f32, tag="lsb", name="lsb")
                    # cross-partition DVE copy: PSUM p64 -> SBUF p0; fast
                    # reciprocal works from SBUF p0. Keeps ACT free for exp.
                    nc.vector.tensor_copy(out=lsb[0:1, cw],
                                          in_=pvts[par][DK:DK + 1, cw])
                    nc.vector.reciprocal_approx_fast(out=rsb[0:1, cw],
                                                     in_=lsb[0:1, cw])
                    nc.gpsimd.partition_broadcast(rb[0:DK, cw], rsb[0:1, cw],
                                                  channels=DK)
                    dst = vals[64 * par:64 * par + 64, hp,
                               j * SC + lo:j * SC + lo + w]
                    if par == 0:
                        nc.vector.tensor_mul(out=dst, in0=pvts[par][0:DK, cw],
                                             in1=rb[0:DK, cw])
                    else:
                        stg = work.tile([P, SC], bf, tag="stg")
                        nc.vector.tensor_mul(out=stg[0:DK, cw],
                                             in0=pvts[par][0:DK, cw],
                                             in1=rb[0:DK, cw])
                        nc.sync.dma_start(out=dst, in_=stg[0:DK, cw])
                if sl is not None:
                    outproj_piece(j, 2 * sl, True)
                    outproj_piece(j, 2 * sl + 1, True)

        # chunk 0 projections up front (256-col halves: rope chains — the
        # startup critical path — begin after 4 instead of 8 matmuls)
        for hf in range(2):
            proj_qk_piece(wq_sb, 0, 0, qts[0], half=hf)
            proj_qk_piece(wk_sb, 0, 0, kts[0], half=hf)
            proj_qk_piece(wq_sb, 1, 0, qts[1], half=hf)
            proj_qk_piece(wk_sb, 1, 0, kts[1], half=hf)
        for s4 in range(4):
            proj_v_piece(0, s4)

        # outproj(j) blocks: half during column j+1, half during j+2 (the
        # later, longer columns have more exp time to hide PE filler under)
        def op_pieces(j, s4s):
            return [lambda j=j, s4=s4: outproj_piece(j, s4, False) for s4 in s4s]

        for sc in range(NSC):
            filler = []
            if sc + 1 < NSC:
                filler.append(lambda sc=sc: proj_qk_piece(wq_sb, 0, sc + 1, qts[0]))
                filler.append(lambda sc=sc: proj_qk_piece(wk_sb, 0, sc + 1, kts[0]))
                filler.append(lambda sc=sc: proj_qk_piece(wq_sb, 1, sc + 1, qts[1]))
                filler.append(lambda sc=sc: proj_qk_piece(wk_sb, 1, sc + 1, kts[1]))
                for s4 in range(4):
                    filler.append(lambda sc=sc, s4=s4: proj_v_piece(sc + 1, s4))
            if sc == 1:
                filler += op_pieces(0, [0, 1])
            elif sc == 2:
                filler += op_pieces(0, [2, 3]) + op_pieces(1, [0, 1])
            elif sc == 3:
                filler += op_pieces(1, [2, 3]) + op_pieces(2, [0, 1, 2, 3])
            ngroups = 2 * (4 * sc + 4)
            nf = len(filler)
            fe = max(1, ngroups // max(nf, 1))
            f0, f1 = filler[:(nf + 1) // 2], filler[(nf + 1) // 2:]
            attention_col(sc, 0, f0, fe)
            attention_col(sc, 1, f1, fe, tail=(sc == NSC - 1))
            for f in f0 + f1:
                f()
            f0.clear()
            f1.clear()

    nc.compile()
    return nc


def get_nc():
    if "nc" not in _CACHE:
        _CACHE["nc"] = _build_nc()
    return _CACHE["nc"]


def make_in_maps(x, Wq, Wk, Wv, Wo, token_positions, rope_theta):
    """Host-side sharding: per-core input dict (bf16, pre-transposed/permuted)."""
    x = np.asarray(x, np.float32)
    Wq = np.asarray(Wq, np.float32)
    Wk = np.asarray(Wk, np.float32)
    Wv = np.asarray(Wv, np.float32)
    Wo = np.asarray(Wo, np.float32)
    pos = np.asarray(token_positions).astype(np.float32)
    theta = float(np.asarray(rope_theta))

    # RoPE row layout per head (64 rows = 2 SBUF quadrants of 32):
    # quadrant q holds [evens of pairs 16q..16q+15, odds of same pairs], so
    # the rotate-half partner is 16 rows away inside the same quadrant.
    perm = []
    sign = np.empty(DK, np.float32)
    pairidx = np.empty(DK, np.int64)
    r = 0
    for q in range(2):
        for p in range(16 * q, 16 * q + 16):
            perm.append(2 * p)
            sign[r] = -1.0
            pairidx[r] = p
            r += 1
        for p in range(16 * q, 16 * q + 16):
            perm.append(2 * p + 1)
            sign[r] = 1.0
            pairidx[r] = p
            r += 1
    perm = np.array(perm)

    freqs = theta ** (-np.arange(DK // 2, dtype=np.float32) / (DK // 2))
    ang = pos[:, None] * freqs[None, :]          # [S, 32]
    cos_t = np.cos(ang).T.astype(np.float32)     # [32, S]
    sin_t = np.sin(ang).T.astype(np.float32)
    cs64 = cos_t[pairidx]                        # [64, S]
    sn64 = sin_t[pairidx] * sign[:, None]        # [64, S]
    # sn multiplies the pre-shuffle tensor: shuffle(q0*sn_pre) == swap(q0)*sn
    sig = np.arange(DK)
    sig = (sig // 32) * 32 + ((sig % 32) + 16) % 32
    snp64 = sn64[sig]
    cs_t = np.tile(cs64, (2, 1)).astype(BF)      # [128, S]
    sn_t = np.tile(snp64, (2, 1)).astype(BF)

    tri_t = np.tril(np.ones((P, P), np.float32)).T.astype(BF)  # keep p<=f

    in_maps = []
    for c in range(8):
        b, g = c // 4, c % 4
        hs = slice(g * E, (g + 1) * E)

        def prep_qk(W, scale):
            Wl = W[hs].reshape(HPC, DK, D)[:, perm, :].reshape(E, D) * scale
            return np.ascontiguousarray(Wl.T).astype(BF)

        in_maps.append({
            "xT": np.ascontiguousarray(x[b].T).astype(BF),
            "wq": prep_qk(Wq, 1.0 / np.sqrt(DK)),
            "wk": prep_qk(Wk, 1.0),
            "wv": np.ascontiguousarray(Wv[hs].T).astype(BF),
            "wo": np.ascontiguousarray(Wo[:, hs].T).astype(BF),
            "cs": cs_t, "sn": sn_t, "tri": tri_t,
        })
    return in_maps


def kernel(x, Wq, Wk, Wv, Wo, token_positions, rope_theta):
    nc = get_nc()
    in_maps = make_in_maps(x, Wq, Wk, Wv, Wo, token_positions, rope_theta)
    from concourse.bass_utils import run_bass_kernel_spmd
    r = run_bass_kernel_spmd(nc, in_maps, core_ids=list(range(8)))
    outs = [np.asarray(m["out"], np.float32) for m in r.results]
    full = np.stack([sum(outs[0:4]), sum(outs[4:8])], 0)
    return full.astype(np.float32)
